# revision 31
# baseline (speedup 1.0000x reference)
"""Trainium2 Bass kernel for the butterfly-CNN problem (nn_CNNLayer_30296699306356).

Network (see problem reference): input conv (k=2,s=2, 1->8 ch) + 10 butterfly
conv levels (k=2,s=2, channels double each level, relu, zero biases) + a
per-block dense matmul (1024 blocks of [8,2]) at the end.

Strategy (memory-regime; weights are ~358 MB fp32 dominated by levels 8-10):
  - Levels 5..9 run in bf16 and level 10 in fp8-e3m4 (weights scaled x4 with
    the inverse folded into fea_dense; activations bf16, fp32 PSUM): cuts the
    dominant w10 HBM stream to 8.4 MB/core so it is fully SBUF-resident long
    before it is consumed. Levels in..4 stay fp32 (earliest levels compound
    quantization error the most). Measured rel err vs fp32 ref: ~1.45e-2.
  - Levels in..7 are replicated on all 8 cores. Levels in..4 use an
    "im2col-packed" layout: activations are stored as [128 partitions =
    (wsub, ch), wHi, b] so every matmul is a full 128-wide contraction with a
    single block-structured 128x128 stationary weight.
  - Level 8 shards OUTPUT channels (256/core). Level 9 shards INPUT channels
    to exactly match l8's output shard, so no collective sits between them;
    each core produces a full-size l9 PARTIAL sum. One 8-core AllReduce
    (256 KB bf16) combines the partials; relu is applied after the reduce.
    This removes the replicated 8.4 MB w8 stream of the previous version AND
    keeps a single collective on the critical path, overlapped with the w10
    weight stream.
  - Level 10 shards OUTPUT channels and runs "orientation B" (activations
    stationary, weights moving) at N=512 per matmul, as two sequential
    output-half passes so half 0's relu/einsum/output overlaps half 1's
    matmuls. Level 10's output shard aligns with the fea_dense block shard,
    so no gather is needed after it.
  - Final block einsum is done on the Vector engine (bf16, mult + reduce).

kernel(**inputs) takes the FULL unsharded inputs and returns the FULL output.
"""

import ml_dtypes
import numpy as np

NCORES = 8
B = 16
P = 128
C = 8
NLVL = 10
BF16 = ml_dtypes.bfloat16
E3M4 = ml_dtypes.float8_e3m4
W10_SCALE = 4.0  # w10 quantized as e3m4*4; 1/4 folded into fea_dense

_CACHE = {}


# ---------------------------------------------------------------- host prep

def _host_prep(inputs):
    """Build the per-core input maps (numpy only)."""
    ind = np.ascontiguousarray(np.asarray(inputs["in_data"], np.float32))
    f = {l: np.asarray(inputs[f"f{l}"], np.float32) for l in range(1, NLVL + 1)}
    f0 = np.asarray(inputs["in_filter"], np.float32)     # [2, 1, 8]
    fd = np.asarray(inputs["fea_dense"], np.float32)     # [1024, 8, 2]

    shared = {}
    # r0 [32, 64, 16]: r0[row, wHi, b] = in[b, wHi*32 + row]
    shared["r0"] = np.ascontiguousarray(
        ind[:, :, 0].reshape(B, 64, 32).transpose(2, 1, 0))

    # w0 [32, 128]: rows (2*wsub + k), cols (wsub*8 + co)
    w0 = np.zeros((32, 128), np.float32)
    for wsub in range(16):
        for k in range(2):
            w0[2 * wsub + k, wsub * 8:wsub * 8 + 8] = f0[k, 0, :]
    shared["w0"] = w0

    # packed levels 1..4 stacked: wpk [4, 128, 128]
    wpk = np.zeros((4, 128, 128), np.float32)
    for lvl in range(1, 5):
        cin = 2 ** (lvl - 1) * C
        cout = 2 ** lvl * C
        s_out = (128 // cin) // 2
        for wso in range(s_out):
            for k in range(2):
                wsi = 2 * wso + k
                wpk[lvl - 1, wsi * cin:(wsi + 1) * cin,
                    wso * cout:(wso + 1) * cout] = f[lvl][k]
    shared["wpk"] = wpk

    # w5/w6/w7 mega-packed [128, 10752] bf16 (kt-major per level), one DMA
    w5v = f[5].astype(BF16).reshape(2, 1, 128, 256)
    w6v = f[6].astype(BF16).reshape(2, 2, 128, 512)
    w7v = f[7].astype(BF16).reshape(2, 4, 128, 1024)
    shared["wmid"] = np.ascontiguousarray(np.concatenate([
        w5v.transpose(2, 0, 1, 3).reshape(128, 512),
        w6v.transpose(2, 0, 1, 3).reshape(128, 2048),
        w7v.transpose(2, 0, 1, 3).reshape(128, 8192)], axis=1))

    # f8 OUTPUT-channel shard: [kt=16, 128, 256], kt = k*8 + cit,
    # rows = cin within the 128-wide cit tile, cols = local cout
    f8b = f[8].astype(BF16)
    w8s = [np.ascontiguousarray(
        f8b[:, :, r * 256:(r + 1) * 256].reshape(2, 8, 128, 256)
        .reshape(16, 128, 256)) for r in range(NCORES)]

    # f9 INPUT-channel shard (matches l8's output shard), packed by output-
    # column block so l9 consumes each 1 MB chunk as its DMA lands:
    # [colblk=4, 128, kt=4, 1024]; kt = k*2 + mt, rows = cin within mt tile
    f9b = f[9].astype(BF16)
    w9s = [np.ascontiguousarray(
        f9b[:, r * 256:(r + 1) * 256, :].reshape(2, 2, 128, 4, 1024)
        .transpose(3, 2, 0, 1, 4).reshape(4, 128, 4, 1024))
        for r in range(NCORES)]

    # f10 in e3m4 (scaled x4): [16 chunks, 128, 4, 1024]
    sh = 8192 // NCORES
    w10s = []
    f10q = (f[10] * W10_SCALE).astype(E3M4)
    for r in range(NCORES):
        blk = f10q[:, :, r * sh:(r + 1) * sh]
        v = blk.reshape(2, 8, 4, 128, sh).transpose(0, 1, 3, 2, 4)
        w10s.append(np.ascontiguousarray(v.reshape(16, 128, 4, sh)))

    # fea_dense shard, per-o flattened, tiled over the 16 batch partitions
    # (bf16: feeds a 2x-rate bf16 Vector einsum)
    fds = []
    for r in range(NCORES):
        blk = fd[r * 128:(r + 1) * 128]                    # [128, 8, 2]
        flat = blk.transpose(2, 0, 1).reshape(2, 1024) / W10_SCALE
        fds.append(np.ascontiguousarray(
            np.broadcast_to(flat[None], (B, 2, 1024)).astype(BF16)))

    in_maps = []
    for r in range(NCORES):
        m = dict(shared)
        m["w8"] = w8s[r]
        m["w9"] = w9s[r]
        m["w10"] = w10s[r]
        m["fdt"] = fds[r]
        in_maps.append(m)
    return in_maps


# ---------------------------------------------------------------- bass build

def _build():
    import concourse.bass as bass
    import concourse.mybir as mybir
    import concourse.tile as tile
    from concourse import bacc

    f32 = mybir.dt.float32
    bf16 = mybir.dt.bfloat16
    RELU = mybir.ActivationFunctionType.Relu

    nc = bacc.Bacc("TRN2", target_bir_lowering=False, debug=False,
                   num_devices=NCORES)

    def inp(name, shape, dt=f32):
        return nc.dram_tensor(name, shape, dt, kind="ExternalInput").ap()

    r0 = inp("r0", [32, 64, 16])
    w0 = inp("w0", [32, 128])
    wpk = inp("wpk", [4, 128, 128])
    wmid = inp("wmid", [128, 10752], bf16)
    w8 = inp("w8", [16, 128, 256], bf16)
    w9 = inp("w9", [4, 128, 4, 1024], bf16)
    f8e3 = mybir.dt.float8e3
    w10 = inp("w10", [16, 128, 4, 1024], f8e3)
    fdt = inp("fdt", [B, 2, 1024], bf16)
    out = nc.dram_tensor("out", [B, 128, 2], f32, kind="ExternalOutput").ap()

    with tile.TileContext(nc) as tc:
        with (
            tc.tile_pool(name="const", bufs=1) as constp,
            tc.tile_pool(name="actp", bufs=3) as actp,
            tc.tile_pool(name="bigp", bufs=1) as bigp,
            tc.tile_pool(name="w7p", bufs=1) as w7p,
            tc.tile_pool(name="w10p", bufs=16) as w10p,
            tc.tile_pool(name="psA", bufs=2, space="PSUM") as psA,
            tc.tile_pool(name="psB", bufs=2, space="PSUM") as psB,
            tc.tile_pool(name="psC", bufs=2, space="PSUM") as psC,
            tc.tile_pool(name="dramp", bufs=1, space="DRAM") as dramp,
        ):
            # ---- resident loads
            r0sb = constp.tile([32, 64, 16], f32, name="r0sb")
            nc.sync.dma_start(r0sb[:], r0)
            w0sb = constp.tile([32, 128], f32, name="w0sb")
            nc.sync.dma_start(w0sb[:], w0)
            wpksb = constp.tile([128, 4, 128], f32, name="wpksb")
            nc.sync.dma_start(wpksb[:], wpk.rearrange("l p c -> p l c"))
            wmidsb = w7p.tile([128, 10752], bf16, name="wmidsb")
            nc.sync.dma_start(wmidsb[:], wmid)
            w5sb = wmidsb[:, 0:512].rearrange("p (t c) -> p t c", c=256)
            w6sb = wmidsb[:, 512:2560].rearrange("p (t c) -> p t c", c=512)
            w7sb = wmidsb[:, 2560:10752].rearrange("p (t c) -> p t c", c=1024)
            fdsb = constp.tile([B, 2, 1024], bf16, name="fdsb")
            nc.sync.dma_start(fdsb[:], fdt)
            w8sb = constp.tile([128, 16, 256], bf16, name="w8sb")
            nc.sync.dma_start(w8sb[:], w8.rearrange("t p c -> p t c"))
            # w9 loaded per column-block so l9 consumes chunks as they land
            w9sb = constp.tile([128, 4, 4, 1024], bf16, name="w9sb")
            for cb4 in range(4):
                nc.sync.dma_start(w9sb[:, cb4], w9[cb4])

            # ---- input conv + packed levels 1..4 (all [128, 64, 16])
            xprev = None
            for lvl in range(5):
                # x4 feeds the bf16 level-5 matmul, so cast at the relu
                xn = actp.tile([128, 64, 16], bf16 if lvl == 4 else f32,
                               name=f"x{lvl}", tag="xl")
                for ch in range(2):
                    ps = psA.tile([128, 32, 16], f32, name="psA", tag="psA")
                    if lvl == 0:
                        nc.tensor.matmul(
                            ps[:], w0sb[:], r0sb[:, ch * 32:(ch + 1) * 32, :],
                            start=True, stop=True)
                    else:
                        nc.tensor.matmul(
                            ps[:], wpksb[:, lvl - 1, :],
                            xprev[:, ch * 32:(ch + 1) * 32, :],
                            start=True, stop=True)
                    nc.scalar.activation(
                        xn[:, ch * 32:(ch + 1) * 32, :], ps[:], RELU)
                xprev = xn

            # ---- standard levels (orientation A, weights stationary)
            def std_level(xin, wsb, cin_t, cout_t, w_out, name, out_tile=None):
                # xin [128, cin_t, 2*w_out, 16]; wsb [128, 2*cin_t, co] with
                # kt = k*cin_t + cit; returns [128, cout_t, w_out, 16]
                if out_tile is None:
                    xn = actp.tile([128, cout_t, w_out, 16], bf16,
                                   name=name, tag="xl")
                else:
                    xn = out_tile
                for ct in range(cout_t):
                    ps = psA.tile([128, w_out, 16], f32, name="psA", tag="psA")
                    for cit in range(cin_t):
                        rhs2 = xin[:, cit].rearrange(
                            "p (w two) b -> p two w b", two=2)
                        for k in range(2):
                            nc.tensor.matmul(
                                ps[:],
                                wsb[:, k * cin_t + cit,
                                    ct * 128:(ct + 1) * 128],
                                rhs2[:, k],
                                start=(cit == 0 and k == 0),
                                stop=(cit == cin_t - 1 and k == 1))
                    nc.scalar.activation(xn[:, ct], ps[:], RELU)
                return xn

            x5 = std_level(xprev[:, None], w5sb, 1, 2, 32, "x5")
            x6 = std_level(x5, w6sb, 2, 4, 16, "x6")
            x7 = std_level(x6, w7sb, 4, 8, 8, "x7")

            # ---- level 8 OUTPUT-sharded (256 cout per core)
            x8loc = bigp.tile([128, 2, 4, 16], bf16, name="x8loc",
                              tag="x8loc")
            for mt in range(2):
                ps = psA.tile([128, 4, 16], f32, name="psA", tag="psA")
                for cit in range(8):
                    rhs2 = x7[:, cit].rearrange(
                        "p (w two) b -> p two w b", two=2)
                    for k in range(2):
                        nc.tensor.matmul(
                            ps[:],
                            w8sb[:, k * 8 + cit, mt * 128:(mt + 1) * 128],
                            rhs2[:, k],
                            start=(cit == 0 and k == 0),
                            stop=(cit == 7 and k == 1))
                nc.scalar.activation(x8loc[:, mt], ps[:], RELU)

            # ---- level 9 INPUT-sharded: full-size partial sums, no relu yet.
            # kt-outer loop so each w9 chunk is consumed as its DMA lands.
            p9sb = bigp.tile([128, 32, 2, 16], bf16, name="p9sb", tag="p9sb")
            ar_in = dramp.tile([128, 32, 2, 16], bf16, name="ar_in")
            ar_out = dramp.tile([128, 32, 2, 16], bf16, name="ar_out",
                                addr_space="Shared")
            ps9 = [psB.tile([128, 16, 2, 16], f32, name=f"ps9_{g}", tag="psB")
                   for g in range(2)]
            rhs9 = [x8loc[:, mt].rearrange(
                "p (w two) b -> p two w b", two=2) for mt in range(2)]
            for mtile in range(32):
                cb4, jw = divmod(mtile, 8)
                g, j = divmod(mtile, 16)
                for kt in range(4):
                    k, mt = divmod(kt, 2)
                    nc.tensor.matmul(
                        ps9[g][:, j],
                        w9sb[:, cb4, kt, jw * 128:(jw + 1) * 128],
                        rhs9[mt][:, k],
                        start=(kt == 0), stop=(kt == 3))
            for g in range(2):
                sl = slice(g * 16, (g + 1) * 16)
                nc.scalar.activation(
                    p9sb[:, sl], ps9[g][:],
                    mybir.ActivationFunctionType.Copy)
                nc.sync.dma_start(ar_in[:, sl], p9sb[:, sl])

            # ---- AllReduce the l9 partials, then relu -> full x9
            nc.gpsimd.collective_compute(
                "AllReduce", mybir.AluOpType.add,
                replica_groups=[list(range(NCORES))],
                ins=[ar_in.opt()], outs=[ar_out.opt()])
            x9pre = bigp.tile([128, 32, 2, 16], bf16, name="x9pre",
                              tag="x9pre")
            x9sb = bigp.tile([128, 32, 2, 16], bf16, name="x9sb", tag="x9sb")
            for q in range(4):
                sl = slice(q * 8, (q + 1) * 8)
                nc.sync.dma_start(x9pre[:, sl], ar_out[:, sl])
                nc.scalar.activation(x9sb[:, sl], x9pre[:, sl], RELU)

            # ---- level 10 (1024-ch shard, orientation B: acts stationary).
            # All 16 e3m4 chunks are SBUF-resident; the two output halves run
            # as sequential PE passes so half 0's relu/einsum/output overlaps
            # half 1's matmuls.
            w10cs = []
            for m in range(16):
                w10c = w10p.tile([128, 4, 1024], f8e3, name="w10c", tag="w10c")
                nc.sync.dma_start(w10c[:], w10[m])
                w10cs.append(w10c)
            ps10 = [psC.tile([B, 512], f32, name=f"ps10_{cb}", tag="psC")
                    for cb in range(2)]
            x10 = bigp.tile([B, 1024], bf16, name="x10")
            osb = bigp.tile([B, 128, 2], f32, name="osb")
            prods = [bigp.tile([B, 1024], bf16, name=f"prod{o}", tag=f"prod{o}")
                     for o in range(2)]
            for cb in range(2):
                csl = slice(cb * 512, (cb + 1) * 512)
                bsl = slice(cb * 64, (cb + 1) * 64)
                for m in range(16):
                    k, q = divmod(m, 8)
                    for j in range(4):
                        t = q * 4 + j
                        nc.tensor.matmul(
                            ps10[cb][:], x9sb[:, t, k, :],
                            w10cs[m][:, j, csl],
                            start=(m == 0 and j == 0),
                            stop=(m == 15 and j == 3))
                nc.scalar.activation(x10[:, csl], ps10[cb][:], RELU)
                # per-block einsum on the vector engine (bf16 2x rate)
                for o in range(2):
                    nc.vector.tensor_tensor(
                        prods[o][:, csl], x10[:, csl], fdsb[:, o, csl],
                        mybir.AluOpType.mult)
                    nc.vector.tensor_reduce(
                        osb[:, bsl, o],
                        prods[o][:, csl].rearrange("p (k c) -> p k c", c=8),
                        mybir.AxisListType.X, mybir.AluOpType.add)
                nc.sync.dma_start(out[:, bsl], osb[:, bsl])

    nc.compile()
    return nc


# ------------------------------------------------------------------- kernel

def kernel(**inputs):
    from concourse.bass_utils import run_bass_kernel_spmd

    in_maps = _host_prep(inputs)
    if "nc" not in _CACHE:
        _CACHE["nc"] = _build()
    nc = _CACHE["nc"]
    res = run_bass_kernel_spmd(nc, in_maps, core_ids=list(range(NCORES)))
    parts = [res.results[r]["out"] for r in range(NCORES)]  # each [16, 128, 2]
    full = np.concatenate(parts, axis=1)                    # [16, 1024, 2]
    return np.ascontiguousarray(full.reshape(B, 2048, 1).astype(np.float32))

